# revision 1
# baseline (speedup 1.0000x reference)
"""Trainium2 Bass kernel for nn_LocalAttention (Luong local attention, N=64, L=H=1024).

Strategy
--------
Data-parallel over batch: 8 batches per NeuronCore x 8 cores.

Host-side layout prep (no model FLOPs on host):
  * For each batch n, p_t = max(src_len - time_step, -1). The Gaussian
    exp(-(l-p_t)^2/25) underflows to exactly 0.0f for |l-p_t| > 51, so the
    context reduction only needs a 128-wide window around p_t.
  * We ROLL each batch's source axis so that window lands at static slots
    [0, 128). Softmax (max/sum) is permutation-invariant, so scores/softmax
    computed in rolled coordinates are exact. Host passes rolled, transposed
    E^T (h on partitions) so the PE can contract over h for scores.
  * W_c is passed transposed (d on partitions) for the output projection.

Device per core (all fp32):
  qa^T = W_a^T @ output^T                      (PE, once)
  per batch b:
    scores = qa_b . E_b^T                      (PE streams E^T, contract h)
    window transpose of E^T[:, 0:128] -> E_win (PE transpose)
    softmax on scores (1,1024) @ partition 0   (DVE max / ACT exp+sum / DVE)
    w = softmax * gauss / Z                    (DVE, one fused op)
    w^T via K=1 matmul with ones               (PE)
    context^T = E_win^T-chunks @ w^T           (PE, 8 tiny matmuls)
  OUT = tanh([context; output] @ W_c^T)        (PE batched over 8, ACT tanh)
"""

import os
import sys

import numpy as np

for _p in ("/opt/trn_rl_repo", "/root/.axon_site/_ro/trn_rl_repo"):
    if os.path.isdir(_p) and _p not in sys.path:
        sys.path.insert(0, _p)

N, L, H = 64, 1024, 1024
NCORES = 8
NB = N // NCORES  # batches per core
WIN = 128         # static window width after roll
DEV_POW = 25.0
KC = H // 128     # 8 contraction chunks

_PROGRAM = None


def _build_program():
    import concourse.tile as tile
    from concourse import bacc, mybir
    from concourse.bass import MemorySpace, ts
    from concourse.masks import make_identity
    from contextlib import ExitStack

    F32 = mybir.dt.float32
    F32R = mybir.dt.float32r  # single-pass fp32 matmul: 4x faster PE, reduced mantissa
    AF = mybir.ActivationFunctionType
    ALU = mybir.AluOpType
    # DT is the dtype of every matmul-operand tensor (DRAM + SBUF); PSUM
    # accumulators and the softmax pipeline stay full fp32.
    DT = F32R if os.environ.get("KERNEL_F32R", "0") == "1" else F32

    nc = bacc.Bacc("TRN2", target_bir_lowering=False, debug=False, num_devices=NCORES)
    # eT pre-interleaved on host: [b, half, p, c*L+l] = E^T[b][512*half+128*c+p, l]
    # so every DMA is one contiguous 16KB read per partition.
    eT = nc.dram_tensor("eT", [NB, 2, 128, (KC // 2) * L], DT, kind="ExternalInput").ap()
    gauss = nc.dram_tensor("gauss", [NB, L], F32, kind="ExternalInput").ap()
    outT = nc.dram_tensor("outT", [H, NB], F32, kind="ExternalInput").ap()
    wa = nc.dram_tensor("wa", [128, KC, H], F32, kind="ExternalInput").ap()
    wcT = nc.dram_tensor("wcT", [128, 2 * KC, H], DT, kind="ExternalInput").ap()
    res = nc.dram_tensor("res", [NB, H], F32, kind="ExternalOutput").ap()

    with tile.TileContext(nc) as tc, ExitStack() as ctx:
        consts = ctx.enter_context(tc.tile_pool(name="consts", bufs=1))
        etp = ctx.enter_context(tc.tile_pool(name="etp", bufs=2))
        work = ctx.enter_context(tc.tile_pool(name="work", bufs=2))
        ps_s = ctx.enter_context(
            tc.tile_pool(name="ps_s", bufs=2, space=MemorySpace.PSUM)
        )
        ps_w = ctx.enter_context(
            tc.tile_pool(name="ps_w", bufs=1, space=MemorySpace.PSUM)
        )
        ps_m = ctx.enter_context(
            tc.tile_pool(name="ps_m", bufs=2, space=MemorySpace.PSUM)
        )

        # ---- constants / weights ----
        wa_sb = consts.tile([128, KC, H], F32)
        nc.sync.dma_start(wa_sb[:], wa[:])
        wcT_sb = consts.tile([128, 2 * KC, H], DT)
        nc.sync.dma_start(wcT_sb[:], wcT[:])
        outT_sb = consts.tile([128, KC, NB], F32)
        nc.sync.dma_start(outT_sb[:], outT.rearrange("(c p) b -> p c b", p=128))
        ident = consts.tile([128, 128], F32)
        make_identity(nc, ident[:])
        ones1 = consts.tile([1, 1], F32)
        nc.gpsimd.memset(ones1[:], 1.0)
        # f32r twin of outT for the final projection (lhsT dtype must match rhs)
        outTr_sb = consts.tile([128, KC, NB], DT)
        if DT is F32:
            outTr_sb = outT_sb
        else:
            nc.vector.tensor_copy(outTr_sb[:], outT_sb[:])
        qaT_sb = consts.tile([128, KC, NB], DT)
        ctxAll = consts.tile([128, KC, NB], DT)

        # ---- qa^T = W_a^T @ output^T : chunk mo of h_out on partitions ----
        for mo in range(KC):
            ps_qa = ps_m.tile([128, NB], F32, tag="misc")
            for c in range(KC):
                nc.tensor.matmul(
                    ps_qa[:],
                    wa_sb[:, c, ts(mo, 128)],
                    outT_sb[:, c, :],
                    start=(c == 0),
                    stop=(c == KC - 1),
                )
            nc.vector.tensor_copy(qaT_sb[:, mo, :], ps_qa[:])

        # ---- per-batch pipeline ----
        HKC = KC // 2  # h-chunks per half-tile
        for b in range(NB):
            ps_scores = ps_s.tile([1, L], F32, tag="scores")
            ps_win = ps_w.tile([128, H], F32, tag="win")
            gauss_b = work.tile([1, L], F32, tag="gauss")
            nc.sync.dma_start(gauss_b[:], gauss[b][None])
            ews = []
            for half in range(2):
                et = etp.tile([128, HKC, L], DT, tag="et")
                nc.sync.dma_start(et[:], eT[b, half].rearrange("p (c l) -> p c l", l=L))
                for cc in range(HKC):
                    c = half * HKC + cc
                    for hh in range(2):
                        nc.tensor.matmul(
                            ps_scores[:, ts(hh, 512)],
                            qaT_sb[:, c, b : b + 1],
                            et[:, cc, ts(hh, 512)],
                            start=(c == 0),
                            stop=(c == KC - 1),
                        )
                    nc.tensor.transpose(
                        ps_win[:, ts(c, 128)], et[:, cc, 0:WIN].bitcast(F32), ident[:]
                    )

            negmax = work.tile([1, 1], F32, tag="negmax")
            nc.vector.reduce_max(
                negmax[:], ps_scores[:], axis=mybir.AxisListType.X, negate=True
            )
            expv = work.tile([1, L], F32, tag="expv")
            zsum = work.tile([1, 1], F32, tag="zsum")
            nc.scalar.activation(
                expv[:], ps_scores[:], AF.Exp, bias=negmax[:], accum_out=zsum[:]
            )
            rz = work.tile([1, 1], F32, tag="rz")
            nc.vector.reciprocal(rz[:], zsum[:])
            wv = work.tile([1, L], F32, tag="wv")
            nc.vector.scalar_tensor_tensor(
                wv[:],
                expv[:],
                rz[:],
                gauss_b[:],
                op0=ALU.mult,
                op1=ALU.mult,
            )
            ew = work.tile([128, H], F32, tag="ew")
            nc.vector.tensor_copy(ew[:], ps_win[:])
            # w^T (window only) via K=1 matmul against ones: out = wv[0,0:128]^T
            ps_wT = ps_m.tile([128, 1], F32, tag="misc")
            nc.tensor.matmul(
                ps_wT[:], wv[:, 0:WIN], ones1[:], start=True, stop=True
            )
            wT_sb = work.tile([128, 1], F32, tag="wT")
            nc.vector.tensor_copy(wT_sb[:], ps_wT[:])
            # context^T chunks: (128 l, 128 h-chunk)^T @ w^T -> (128 h, 1)
            ps_ctx = ps_m.tile([128, NB], F32, tag="misc")
            for c in range(KC):
                nc.tensor.matmul(
                    ps_ctx[:, c : c + 1],
                    ew[:, ts(c, 128)],
                    wT_sb[:],
                    start=True,
                    stop=True,
                )
            nc.vector.tensor_copy(ctxAll[:, :, b], ps_ctx[:])

        # ---- OUT = tanh(cat @ W_c^T), batched over the core's 8 rows ----
        res_sb = work.tile([NB, H], F32, tag="res")
        for hh in range(2):
            ps_out = ps_m.tile([NB, 512], F32, tag="misc")
            for d in range(2 * KC):
                lhsT = ctxAll[:, d, :] if d < KC else outTr_sb[:, d - KC, :]
                nc.tensor.matmul(
                    ps_out[:],
                    lhsT,
                    wcT_sb[:, d, ts(hh, 512)],
                    start=(d == 0),
                    stop=(d == 2 * KC - 1),
                )
            nc.scalar.activation(res_sb[:, ts(hh, 512)], ps_out[:], AF.Tanh)
        nc.sync.dma_start(res[:], res_sb[:])

    nc.compile()
    return nc


def _get_program():
    global _PROGRAM
    if _PROGRAM is None:
        _PROGRAM = _build_program()
    return _PROGRAM


def _prepare(inputs):
    E = np.asarray(inputs["encoder_outputs"], dtype=np.float32)
    out = np.asarray(inputs["output"], dtype=np.float32).reshape(N, H)
    W_a = np.ascontiguousarray(np.asarray(inputs["W_a"], dtype=np.float32))
    W_c = np.asarray(inputs["W_c"], dtype=np.float32)
    src_len = np.asarray(inputs["src_len"]).reshape(N).astype(np.int64)
    t = int(np.asarray(inputs["time_step"]))

    p_t = np.maximum(src_len - t, -1)
    roll = p_t - (WIN // 2 - 1)  # window slot j <-> original l = (j + roll) % L
    j = np.arange(L, dtype=np.int64)
    idx = (j[None, :] + roll[:, None]) % L  # (N, L)
    ptf = p_t.astype(np.float32)[:, None]
    gauss = np.exp(
        -((idx.astype(np.float32) - ptf) ** 2) / np.float32(DEV_POW)
    ).astype(np.float32)

    Er = E[np.arange(N)[:, None], idx, :]  # (N, L, H) rolled
    eT = np.ascontiguousarray(Er.transpose(0, 2, 1))  # (N, H, L)
    # interleave for linear per-partition DMA: [n, half, p, c, l] = eT[n, 512h+128c+p, l]
    eT_dev = np.ascontiguousarray(
        eT.reshape(N, 2, KC // 2, 128, L).transpose(0, 1, 3, 2, 4)
    ).reshape(N, 2, 128, (KC // 2) * L)
    wa_dev = np.ascontiguousarray(
        W_a.reshape(KC, 128, H).transpose(1, 0, 2)
    )  # (128, KC, H)
    wcT = np.ascontiguousarray(W_c.T)  # (2H, H)
    wcT_dev = np.ascontiguousarray(
        wcT.reshape(2 * KC, 128, H).transpose(1, 0, 2)
    )  # (128, 2KC, H)

    in_maps = []
    for c in range(NCORES):
        sl = slice(c * NB, (c + 1) * NB)
        in_maps.append(
            {
                "eT": eT_dev[sl],
                "gauss": np.ascontiguousarray(gauss[sl]),
                "outT": np.ascontiguousarray(out[sl].T),
                "wa": wa_dev,
                "wcT": wcT_dev,
            }
        )
    return in_maps


def _run(inputs, trace=False, tmpdir=None):
    from concourse.bass_utils import run_bass_kernel_spmd

    nc = _get_program()
    in_maps = _prepare(inputs)
    r = run_bass_kernel_spmd(
        nc, in_maps, core_ids=list(range(NCORES)), trace=trace, tmpdir=tmpdir
    )
    outp = np.concatenate([r.results[c]["res"] for c in range(NCORES)], axis=0)
    return np.ascontiguousarray(outp.reshape(N, 1, H).astype(np.float32)), r


def kernel(**inputs):
    return _run(inputs, trace=False)[0]



# revision 5
# speedup vs baseline: 1.6396x; 1.6396x over previous
"""Trainium2 Bass kernel for nn_LocalAttention (Luong local attention, N=64, L=H=1024).

Strategy
--------
Data-parallel over batch: 8 batches per NeuronCore x 8 cores.

Host-side layout prep (no model FLOPs on host):
  * For each batch n, p_t = max(src_len - time_step, -1). The Gaussian
    exp(-(l-p_t)^2/25) underflows to exactly 0.0f for |l-p_t| > 51, so the
    context reduction only needs a 128-wide window around p_t.
  * We ROLL each batch's source axis so that window lands at static slots
    [0, 128). Softmax (max/sum) is permutation-invariant, so scores/softmax
    computed in rolled coordinates are exact. Host passes rolled, transposed
    E^T (h on partitions) so the PE can contract over h for scores.
  * W_c is passed transposed (d on partitions) for the output projection.

Precision plan (tolerance is 2e-2 relative; fp16 ops accumulate in fp32 PSUM):
  * E and W_c stream as fp16 (1 PE cycle/row instead of 4 for fp32; half DMA).
  * qa = W_a^T h is computed in full fp32 (its error is amplified by sqrt(H)
    in the scores dot), then rounded to fp16 for the scores matmul.
  * Softmax pipeline (max/exp/sum/normalize) stays fp32.

Device per core (PSUM fp32 accumulation everywhere):
  qa^T = W_a^T @ output^T                      (PE fp32, once; out free=8 so cheap)
  per batch b:
    scores = qa_b . E_b^T                      (PE fp16, streams E^T, contract h)
    window transpose of E^T[:, 0:128] -> E_win (PE fp16 transpose)
    softmax on scores (1,1024) @ partition 0   (DVE max / ACT exp+sum / DVE)
    w = softmax * gauss / Z                    (DVE, one fused op)
    w^T via K=1 matmul with ones               (PE)
    context^T = E_win^T-chunks @ w^T           (PE fp16, 8 tiny matmuls)
  OUT = tanh([context; output] @ W_c^T)        (PE fp16 batched over 8, ACT tanh)
"""

import os
import sys

import numpy as np

for _p in ("/opt/trn_rl_repo", "/root/.axon_site/_ro/trn_rl_repo"):
    if os.path.isdir(_p) and _p not in sys.path:
        sys.path.insert(0, _p)

N, L, H = 64, 1024, 1024
NCORES = 8
NB = N // NCORES  # batches per core
WIN = 128         # static window width after roll
DEV_POW = 25.0
KC = H // 128     # 8 contraction chunks

_PROGRAM = None


def _build_program():
    import concourse.tile as tile
    from concourse import bacc, mybir
    from concourse.bass import MemorySpace, ts
    from concourse.masks import make_identity
    from contextlib import ExitStack

    F32 = mybir.dt.float32
    F16 = mybir.dt.float16
    AF = mybir.ActivationFunctionType
    ALU = mybir.AluOpType

    nc = bacc.Bacc("TRN2", target_bir_lowering=False, debug=False, num_devices=NCORES)
    # eT pre-interleaved on host: [b, half, p, c*L+l] = E^T[b][512*half+128*c+p, l]
    # so every DMA is one contiguous 8KB read per partition.
    eT = nc.dram_tensor("eT", [NB, 2, 128, (KC // 2) * L], F16, kind="ExternalInput").ap()
    gauss = nc.dram_tensor("gauss", [NB, L], F32, kind="ExternalInput").ap()
    outT = nc.dram_tensor("outT", [H, NB], F32, kind="ExternalInput").ap()
    wa = nc.dram_tensor("wa", [128, KC, H], F32, kind="ExternalInput").ap()
    wcT = nc.dram_tensor("wcT", [128, 2 * KC, H], F16, kind="ExternalInput").ap()
    res = nc.dram_tensor("res", [NB, H], F32, kind="ExternalOutput").ap()

    with tile.TileContext(nc) as tc, ExitStack() as ctx:
        consts = ctx.enter_context(tc.tile_pool(name="consts", bufs=1))
        etp = ctx.enter_context(tc.tile_pool(name="etp", bufs=2))
        work = ctx.enter_context(tc.tile_pool(name="work", bufs=2))
        ps_s = ctx.enter_context(
            tc.tile_pool(name="ps_s", bufs=2, space=MemorySpace.PSUM)
        )
        ps_w = ctx.enter_context(
            tc.tile_pool(name="ps_w", bufs=1, space=MemorySpace.PSUM)
        )
        ps_m = ctx.enter_context(
            tc.tile_pool(name="ps_m", bufs=2, space=MemorySpace.PSUM)
        )

        # ---- constants / weights ----
        wa_sb = consts.tile([128, KC, H], F32)
        nc.sync.dma_start(wa_sb[:], wa[:])
        wcT_sb = consts.tile([128, 2 * KC, H], F16)
        nc.sync.dma_start(wcT_sb[:], wcT[:])
        outT_sb = consts.tile([128, KC, NB], F32)
        nc.sync.dma_start(outT_sb[:], outT.rearrange("(c p) b -> p c b", p=128))
        ident = consts.tile([128, 128], F16)
        make_identity(nc, ident[:])
        ones1 = consts.tile([1, 1], F16)
        nc.gpsimd.memset(ones1[:], 1.0)
        # fp16 twin of outT for the final projection (lhsT dtype must match rhs)
        outTr_sb = consts.tile([128, KC, NB], F16)
        nc.vector.tensor_copy(outTr_sb[:], outT_sb[:])
        qaT_sb = consts.tile([128, KC, NB], F16)
        ctxAll = consts.tile([128, KC, NB], F16)

        # ---- qa^T = W_a^T @ output^T : chunk mo of h_out on partitions ----
        for mo in range(KC):
            ps_qa = ps_m.tile([128, NB], F32, tag="misc")
            for c in range(KC):
                nc.tensor.matmul(
                    ps_qa[:],
                    wa_sb[:, c, ts(mo, 128)],
                    outT_sb[:, c, :],
                    start=(c == 0),
                    stop=(c == KC - 1),
                )
            nc.vector.tensor_copy(qaT_sb[:, mo, :], ps_qa[:])

        # ---- per-batch pipeline ----
        HKC = KC // 2  # h-chunks per half-tile
        for b in range(NB):
            ps_scores = ps_s.tile([1, L], F32, tag="scores")
            ps_win = ps_w.tile([128, H], F16, tag="win")
            gauss_b = work.tile([1, L], F32, tag="gauss")
            nc.sync.dma_start(gauss_b[:], gauss[b][None])
            for half in range(2):
                et = etp.tile([128, HKC, L], F16, tag="et")
                nc.sync.dma_start(et[:], eT[b, half].rearrange("p (c l) -> p c l", l=L))
                for cc in range(HKC):
                    c = half * HKC + cc
                    for hh in range(2):
                        nc.tensor.matmul(
                            ps_scores[:, ts(hh, 512)],
                            qaT_sb[:, c, b : b + 1],
                            et[:, cc, ts(hh, 512)],
                            start=(c == 0),
                            stop=(c == KC - 1),
                        )
                    nc.tensor.transpose(
                        ps_win[:, ts(c, 128)], et[:, cc, 0:WIN], ident[:]
                    )

            negmax = work.tile([1, 1], F32, tag="negmax")
            nc.vector.reduce_max(
                negmax[:], ps_scores[:], axis=mybir.AxisListType.X, negate=True
            )
            expv = work.tile([1, L], F32, tag="expv")
            zsum = work.tile([1, 1], F32, tag="zsum")
            nc.scalar.activation(
                expv[:], ps_scores[:], AF.Exp, bias=negmax[:], accum_out=zsum[:]
            )
            rz = work.tile([1, 1], F32, tag="rz")
            nc.vector.reciprocal(rz[:], zsum[:])
            wv = work.tile([1, L], F32, tag="wv")
            nc.vector.scalar_tensor_tensor(
                wv[:],
                expv[:],
                rz[:],
                gauss_b[:],
                op0=ALU.mult,
                op1=ALU.mult,
            )
            ew = work.tile([128, H], F16, tag="ew")
            nc.vector.tensor_copy(ew[:], ps_win[:])
            # w^T (window only) via K=1 matmul with ones: out = wv[0,0:128]^T
            wv16 = work.tile([1, WIN], F16, tag="wv16")
            nc.vector.tensor_copy(wv16[:], wv[:, 0:WIN])
            ps_wT = ps_m.tile([128, 1], F32, tag="misc")
            nc.tensor.matmul(
                ps_wT[:], wv16[:], ones1[:], start=True, stop=True
            )
            wT_sb = work.tile([128, 1], F16, tag="wT")
            nc.vector.tensor_copy(wT_sb[:], ps_wT[:])
            # context^T chunks: (128 l, 128 h-chunk)^T @ w^T -> (128 h, 1)
            ps_ctx = ps_m.tile([128, NB], F32, tag="misc")
            for c in range(KC):
                nc.tensor.matmul(
                    ps_ctx[:, c : c + 1],
                    ew[:, ts(c, 128)],
                    wT_sb[:],
                    start=True,
                    stop=True,
                )
            nc.vector.tensor_copy(ctxAll[:, :, b], ps_ctx[:])

        # ---- OUT = tanh(cat @ W_c^T), batched over the core's 8 rows ----
        res_sb = work.tile([NB, H], F32, tag="res")
        for hh in range(2):
            ps_out = ps_m.tile([NB, 512], F32, tag="misc")
            for d in range(2 * KC):
                lhsT = ctxAll[:, d, :] if d < KC else outTr_sb[:, d - KC, :]
                nc.tensor.matmul(
                    ps_out[:],
                    lhsT,
                    wcT_sb[:, d, ts(hh, 512)],
                    start=(d == 0),
                    stop=(d == 2 * KC - 1),
                )
            nc.scalar.activation(res_sb[:, ts(hh, 512)], ps_out[:], AF.Tanh)
        nc.sync.dma_start(res[:], res_sb[:])

    nc.compile()
    return nc


def _get_program():
    global _PROGRAM
    if _PROGRAM is None:
        _PROGRAM = _build_program()
    return _PROGRAM


def _prepare(inputs):
    E = np.asarray(inputs["encoder_outputs"], dtype=np.float32)
    out = np.asarray(inputs["output"], dtype=np.float32).reshape(N, H)
    W_a = np.ascontiguousarray(np.asarray(inputs["W_a"], dtype=np.float32))
    W_c = np.asarray(inputs["W_c"], dtype=np.float32)
    src_len = np.asarray(inputs["src_len"]).reshape(N).astype(np.int64)
    t = int(np.asarray(inputs["time_step"]))

    p_t = np.maximum(src_len - t, -1)
    roll = p_t - (WIN // 2 - 1)  # window slot j <-> original l = (j + roll) % L
    j = np.arange(L, dtype=np.int64)
    idx = (j[None, :] + roll[:, None]) % L  # (N, L)
    ptf = p_t.astype(np.float32)[:, None]
    gauss = np.exp(
        -((idx.astype(np.float32) - ptf) ** 2) / np.float32(DEV_POW)
    ).astype(np.float32)

    Er = E[np.arange(N)[:, None], idx, :]  # (N, L, H) rolled
    eT = np.ascontiguousarray(Er.transpose(0, 2, 1)).astype(np.float16)  # (N, H, L)
    # interleave for linear per-partition DMA: [n, half, p, c, l] = eT[n, 512h+128c+p, l]
    eT_dev = np.ascontiguousarray(
        eT.reshape(N, 2, KC // 2, 128, L).transpose(0, 1, 3, 2, 4)
    ).reshape(N, 2, 128, (KC // 2) * L)
    wa_dev = np.ascontiguousarray(
        W_a.reshape(KC, 128, H).transpose(1, 0, 2)
    )  # (128, KC, H)
    wcT = np.ascontiguousarray(W_c.T)  # (2H, H)
    wcT_dev = np.ascontiguousarray(
        wcT.reshape(2 * KC, 128, H).transpose(1, 0, 2)
    ).astype(np.float16)  # (128, 2KC, H)

    in_maps = []
    for c in range(NCORES):
        sl = slice(c * NB, (c + 1) * NB)
        in_maps.append(
            {
                "eT": eT_dev[sl],
                "gauss": np.ascontiguousarray(gauss[sl]),
                "outT": np.ascontiguousarray(out[sl].T),
                "wa": wa_dev,
                "wcT": wcT_dev,
            }
        )
    return in_maps


def _run(inputs, trace=False, tmpdir=None):
    from concourse.bass_utils import run_bass_kernel_spmd

    nc = _get_program()
    in_maps = _prepare(inputs)
    r = run_bass_kernel_spmd(
        nc, in_maps, core_ids=list(range(NCORES)), trace=trace, tmpdir=tmpdir
    )
    outp = np.concatenate([r.results[c]["res"] for c in range(NCORES)], axis=0)
    return np.ascontiguousarray(outp.reshape(N, 1, H).astype(np.float32)), r


def kernel(**inputs):
    return _run(inputs, trace=False)[0]


# revision 16
# speedup vs baseline: 2.0696x; 1.2622x over previous
"""Trainium2 Bass kernel for nn_LocalAttention (Luong local attention, N=64, L=H=1024).

Strategy
--------
Data-parallel over batch: 8 batches per NeuronCore x 8 cores.

Host-side layout prep (no model FLOPs on host):
  * For each batch n, p_t = max(src_len - time_step, -1). The Gaussian
    exp(-(l-p_t)^2/25) underflows to exactly 0.0f for |l-p_t| > 51, so the
    context reduction only needs a 128-wide window around p_t.
  * We ROLL each batch's source axis so that window lands at static slots
    [0, 128). Softmax (max/sum) is permutation-invariant, so scores/softmax
    computed in rolled coordinates are exact. Host passes rolled, transposed
    E^T (h on partitions) for the scores contraction, plus the first 128
    rolled rows as-is (eWin, l on partitions) for the context contraction.
  * W_a is passed as an fp16 hi/lo pair (W = hi + 2^-11 * lo) so qa keeps
    ~22 mantissa bits: qa errors are amplified by sqrt(H) in the scores dot,
    so plain fp16 W_a would be too coarse. W_c is fp16 (tolerance 2e-2).

Device per core (PSUM accumulates fp32 everywhere):
  qa rows = outT^T @ [W_a_hi; W_a_lo]          (PE fp16, streams W_a chunks)
  qa^T via 8 tiny PE transposes                (PE fp16)
  per batch b:
    scores = qa_b . E_b^T                      (PE fp16, streams E^T)
    softmax on scores (1,1024) @ partition 0   (DVE max / ACT exp+sum / DVE)
    w = softmax * gauss / Z on window          (DVE, fused, fp16 out)
    w^T via K=1 matmul with ones               (PE)
    context^T = eWin-chunks^T @ w^T            (PE fp16, 8 tiny matmuls)
  OUT = tanh([context; output] @ W_c^T)        (PE fp16 batched over 8, ACT tanh)
"""

import os
import sys

import numpy as np

for _p in ("/opt/trn_rl_repo", "/root/.axon_site/_ro/trn_rl_repo"):
    if os.path.isdir(_p) and _p not in sys.path:
        sys.path.insert(0, _p)

N, L, H = 64, 1024, 1024
NCORES = 8
NB = N // NCORES  # batches per core
WIN = 128         # static window width after roll
DEV_POW = 25.0
KC = H // 128     # 8 contraction chunks
LO_SCALE = 2.0 ** 11

_PROGRAM = None


def _build_program():
    import concourse.tile as tile
    from concourse import bacc, mybir
    from concourse.bass import MemorySpace, ts
    from concourse.masks import make_identity
    from contextlib import ExitStack

    F32 = mybir.dt.float32
    F16 = mybir.dt.float16
    AF = mybir.ActivationFunctionType
    ALU = mybir.AluOpType

    nc = bacc.Bacc("TRN2", target_bir_lowering=False, debug=False, num_devices=NCORES)
    # eT pre-interleaved on host: [b, half, p, c*L+l] = E^T[b][512*half+128*c+p, l]
    # so every DMA is one contiguous 8KB read per partition.
    eT = nc.dram_tensor("eT", [NB, 2, 128, (KC // 2) * L], F16, kind="ExternalInput").ap()
    ewin = nc.dram_tensor("ewin", [NB, WIN, H], F16, kind="ExternalInput").ap()
    gauss = nc.dram_tensor("gauss", [1, NB * WIN], F32, kind="ExternalInput").ap()
    # outT16 = fp16(h); outTlo = fp16(2^-11 h) pairs with wa2's lo plane so the
    # lo partial products accumulate into the same PSUM group as the hi ones.
    outT16 = nc.dram_tensor("outT16", [H, NB], F16, kind="ExternalInput").ap()
    outTlo = nc.dram_tensor("outTlo", [H, NB], F16, kind="ExternalInput").ap()
    # wa2[c] = [128 h_in, {hi,lo}, 1024 h_out] fp16 pair, W_a = hi + 2^-11 lo
    wa2 = nc.dram_tensor("wa2", [KC, 128, 2, H], F16, kind="ExternalInput").ap()
    wcT = nc.dram_tensor("wcT", [128, 2 * KC, H], F16, kind="ExternalInput").ap()
    res = nc.dram_tensor("res", [NB, H], F32, kind="ExternalOutput").ap()

    with tile.TileContext(nc) as tc, ExitStack() as ctx:
        consts = ctx.enter_context(tc.tile_pool(name="consts", bufs=1))
        etp = ctx.enter_context(tc.tile_pool(name="etp", bufs=4))
        ewp = ctx.enter_context(tc.tile_pool(name="ewp", bufs=3))
        wap = ctx.enter_context(tc.tile_pool(name="wap", bufs=8))
        work = ctx.enter_context(tc.tile_pool(name="work", bufs=2))
        ps_s = ctx.enter_context(
            tc.tile_pool(name="ps_s", bufs=2, space=MemorySpace.PSUM)
        )
        ps_q = ctx.enter_context(
            tc.tile_pool(name="ps_q", bufs=1, space=MemorySpace.PSUM)
        )
        ps_m = ctx.enter_context(
            tc.tile_pool(name="ps_m", bufs=2, space=MemorySpace.PSUM)
        )

        # ---- early DMAs: batch 0/1 inputs first so the PE can start ASAP ----
        et_tiles = {}
        for b in range(2):
            for half in range(2):
                t = etp.tile([128, KC // 2, L], F16, tag="et")
                nc.sync.dma_start(t[:], eT[b, half].rearrange("p (c l) -> p c l", l=L))
                et_tiles[(b, half)] = t
        ewin_tiles = {}
        for b in range(2):
            t = ewp.tile([WIN, H], F16, tag="ewin")
            nc.sync.dma_start(t[:], ewin[b])
            ewin_tiles[b] = t
        gauss_sb = consts.tile([1, NB * WIN], F32)
        nc.sync.dma_start(gauss_sb[:], gauss[:])
        outTr_sb = consts.tile([128, KC, NB], F16)
        nc.sync.dma_start(outTr_sb[:], outT16.rearrange("(c p) b -> p c b", p=128))
        outTlo_sb = consts.tile([128, KC, NB], F16)
        nc.sync.dma_start(outTlo_sb[:], outTlo.rearrange("(c p) b -> p c b", p=128))
        wa_tiles = []
        for c in range(KC):
            t = wap.tile([128, 2, H], F16, tag="wa")
            nc.sync.dma_start(t[:], wa2[c])
            wa_tiles.append(t)

        ident = consts.tile([128, 128], F16)
        make_identity(nc, ident[:])
        ones1 = consts.tile([1, 1], F16)
        nc.gpsimd.memset(ones1[:], 1.0)
        qaT_sb = consts.tile([128, KC, NB], F16)
        ctxAll = consts.tile([128, KC, NB], F16)

        # ---- qa rows = h^T W_a: hi and (pre-scaled) lo partial products all
        # accumulate into one fp32 PSUM group ----
        qrow16 = consts.tile([NB, H], F16)
        for hh in range(2):
            ps_qa = ps_q.tile([NB, 512], F32, tag="q")
            for c in range(KC):
                nc.tensor.matmul(
                    ps_qa[:],
                    outTr_sb[:, c, :],
                    wa_tiles[c][:, 0, ts(hh, 512)],
                    start=(c == 0),
                    stop=False,
                )
            for c in range(KC):
                nc.tensor.matmul(
                    ps_qa[:],
                    outTlo_sb[:, c, :],
                    wa_tiles[c][:, 1, ts(hh, 512)],
                    start=False,
                    stop=(c == KC - 1),
                )
            nc.vector.tensor_copy(qrow16[:, ts(hh, 512)], ps_qa[:])
            for cc in range(KC // 2):
                c = hh * (KC // 2) + cc
                ps_t = ps_m.tile([128, NB], F16, tag="misc")
                nc.tensor.transpose(
                    ps_t[:], qrow16[:, ts(c, 128)], ident[0:NB, 0:NB]
                )
                nc.vector.tensor_copy(qaT_sb[:, c, :], ps_t[:])

        # ---- per-batch pipeline ----
        HKC = KC // 2  # h-chunks per half-tile
        for b in range(NB):
            # prefetch batch b+2 inputs (b0/b1 were issued before qa)
            pb = b + 2
            if pb < NB:
                for half in range(2):
                    t = etp.tile([128, HKC, L], F16, tag="et")
                    nc.sync.dma_start(
                        t[:], eT[pb, half].rearrange("p (c l) -> p c l", l=L)
                    )
                    et_tiles[(pb, half)] = t
                t = ewp.tile([WIN, H], F16, tag="ewin")
                nc.sync.dma_start(t[:], ewin[pb])
                ewin_tiles[pb] = t
            if 2 <= b <= 5:
                # wcT is only needed by the tail projection; stream its quarters
                # mid-pipeline so they never delay the per-batch eT stream.
                if b == 2:
                    wcT_sb = consts.tile([128, 2 * KC, H], F16)
                q = b - 2
                nc.sync.dma_start(
                    wcT_sb[:, ts(q, 2 * KC // 4), :], wcT[:, ts(q, 2 * KC // 4), :]
                )

            ps_scores = ps_s.tile([1, L], F32, tag="scores")
            for half in range(2):
                et = et_tiles.pop((b, half))
                for cc in range(HKC):
                    c = half * HKC + cc
                    for hh in range(2):
                        nc.tensor.matmul(
                            ps_scores[:, ts(hh, 512)],
                            qaT_sb[:, c, b : b + 1],
                            et[:, cc, ts(hh, 512)],
                            start=(c == 0),
                            stop=(c == KC - 1),
                        )
            ew = ewin_tiles.pop(b)

            negmax = work.tile([1, 1], F32, tag="negmax")
            nc.vector.reduce_max(
                negmax[:], ps_scores[:], axis=mybir.AxisListType.X, negate=True
            )
            expv = work.tile([1, L], F32, tag="expv")
            zsum = work.tile([1, 1], F32, tag="zsum")
            nc.scalar.activation(
                expv[:], ps_scores[:], AF.Exp, bias=negmax[:], accum_out=zsum[:]
            )
            rz = work.tile([1, 1], F32, tag="rz")
            nc.vector.reciprocal(rz[:], zsum[:])
            # w (window only) = exp * (1/Z) * gauss, rounded to fp16
            wv32 = work.tile([1, WIN], F32, tag="wv32")
            nc.vector.scalar_tensor_tensor(
                wv32[:],
                expv[:, 0:WIN],
                rz[:],
                gauss_sb[:, ts(b, WIN)],
                op0=ALU.mult,
                op1=ALU.mult,
            )
            wv16 = work.tile([1, WIN], F16, tag="wv16")
            nc.vector.tensor_copy(wv16[:], wv32[:])
            # w^T via K=1 matmul with ones: out = wv16^T
            ps_wT = ps_m.tile([128, 1], F32, tag="misc")
            nc.tensor.matmul(ps_wT[:], wv16[:], ones1[:], start=True, stop=True)
            wT_sb = work.tile([128, 1], F16, tag="wT")
            nc.vector.tensor_copy(wT_sb[:], ps_wT[:])
            # context^T chunks: eWin[:, ts(c,128)]^T @ w^T -> (128 h, 1)
            ps_ctx = ps_m.tile([128, NB], F32, tag="misc")
            for c in range(KC):
                nc.tensor.matmul(
                    ps_ctx[:, c : c + 1],
                    ew[:, ts(c, 128)],
                    wT_sb[:],
                    start=True,
                    stop=True,
                )
            nc.vector.tensor_copy(ctxAll[:, :, b], ps_ctx[:])

        # ---- OUT = tanh(cat @ W_c^T), batched over the core's 8 rows ----
        res_sb = work.tile([NB, H], F32, tag="res")
        for hh in range(2):
            ps_out = ps_m.tile([NB, 512], F32, tag="misc")
            for d in range(2 * KC):
                lhsT = ctxAll[:, d, :] if d < KC else outTr_sb[:, d - KC, :]
                nc.tensor.matmul(
                    ps_out[:],
                    lhsT,
                    wcT_sb[:, d, ts(hh, 512)],
                    start=(d == 0),
                    stop=(d == 2 * KC - 1),
                )
            nc.scalar.activation(res_sb[:, ts(hh, 512)], ps_out[:], AF.Tanh)
        nc.sync.dma_start(res[:], res_sb[:])

    nc.compile()
    return nc


def _get_program():
    global _PROGRAM
    if _PROGRAM is None:
        _PROGRAM = _build_program()
    return _PROGRAM


def _prepare(inputs):
    E = np.asarray(inputs["encoder_outputs"], dtype=np.float32)
    out = np.asarray(inputs["output"], dtype=np.float32).reshape(N, H)
    W_a = np.ascontiguousarray(np.asarray(inputs["W_a"], dtype=np.float32))
    W_c = np.asarray(inputs["W_c"], dtype=np.float32)
    src_len = np.asarray(inputs["src_len"]).reshape(N).astype(np.int64)
    t = int(np.asarray(inputs["time_step"]))

    p_t = np.maximum(src_len - t, -1)
    roll = p_t - (WIN // 2 - 1)  # window slot j <-> original l = (j + roll) % L
    j = np.arange(L, dtype=np.int64)
    idx = (j[None, :] + roll[:, None]) % L  # (N, L)
    ptf = p_t.astype(np.float32)[:, None]
    gauss = np.exp(
        -((idx[:, :WIN].astype(np.float32) - ptf) ** 2) / np.float32(DEV_POW)
    ).astype(np.float32)  # (N, WIN)

    Er = E[np.arange(N)[:, None], idx, :]  # (N, L, H) rolled
    ewin_dev = np.ascontiguousarray(Er[:, :WIN, :]).astype(np.float16)  # (N, WIN, H)
    eT = np.ascontiguousarray(Er.transpose(0, 2, 1)).astype(np.float16)  # (N, H, L)
    # interleave for linear per-partition DMA: [n, half, p, c, l] = eT[n, 512h+128c+p, l]
    eT_dev = np.ascontiguousarray(
        eT.reshape(N, 2, KC // 2, 128, L).transpose(0, 1, 3, 2, 4)
    ).reshape(N, 2, 128, (KC // 2) * L)
    # W_a fp16 hi/lo pair: W = hi + 2^-11 * lo  (lo scaled into fp16 normal range)
    wa_hi = W_a.astype(np.float16)
    wa_lo = ((W_a - wa_hi.astype(np.float32)) * np.float32(LO_SCALE)).astype(np.float16)
    # wa2[c, p, {hi,lo}, :] = pair[128c + p, :]
    wa2_dev = np.ascontiguousarray(
        np.stack([wa_hi, wa_lo], axis=1).reshape(KC, 128, 2, H)
    )
    wcT = np.ascontiguousarray(W_c.T)  # (2H, H)
    wcT_dev = np.ascontiguousarray(
        wcT.reshape(2 * KC, 128, H).transpose(1, 0, 2)
    ).astype(np.float16)  # (128, 2KC, H)

    in_maps = []
    for c in range(NCORES):
        sl = slice(c * NB, (c + 1) * NB)
        outT = np.ascontiguousarray(out[sl].T)
        in_maps.append(
            {
                "eT": eT_dev[sl],
                "ewin": ewin_dev[sl],
                "gauss": np.ascontiguousarray(gauss[sl].reshape(1, NB * WIN)),
                "outT16": outT.astype(np.float16),
                "outTlo": (outT / np.float32(LO_SCALE)).astype(np.float16),
                "wa2": wa2_dev,
                "wcT": wcT_dev,
            }
        )
    return in_maps


def _run(inputs, trace=False, tmpdir=None):
    from concourse.bass_utils import run_bass_kernel_spmd

    nc = _get_program()
    in_maps = _prepare(inputs)
    r = run_bass_kernel_spmd(
        nc, in_maps, core_ids=list(range(NCORES)), trace=trace, tmpdir=tmpdir
    )
    outp = np.concatenate([r.results[c]["res"] for c in range(NCORES)], axis=0)
    return np.ascontiguousarray(outp.reshape(N, 1, H).astype(np.float32)), r


def kernel(**inputs):
    return _run(inputs, trace=False)[0]


# revision 20
# speedup vs baseline: 2.5578x; 1.2359x over previous
"""Trainium2 Bass kernel for nn_LocalAttention (Luong local attention, N=64, L=H=1024).

Strategy
--------
Data-parallel over batch: 8 batches per NeuronCore x 8 cores.

Host-side layout prep (no model FLOPs on host):
  * For each batch n, p_t = max(src_len - time_step, -1). The Gaussian
    exp(-(l-p_t)^2/25) underflows to exactly 0.0f for |l-p_t| > 51, so the
    context reduction only needs a 128-wide window around p_t.
  * We ROLL each batch's source axis so that window lands at static slots
    [0, 128). Softmax (max/sum) is permutation-invariant, so scores/softmax
    computed in rolled coordinates are exact. Host passes rolled, transposed
    E^T (h on partitions) for the scores contraction, plus the first 128
    rolled rows as-is (eWin, l on partitions) for the context contraction.
  * W_a is passed as an fp16 hi/lo pair (W ~= hi + 2^-11 lo to ~22 mantissa
    bits): qa errors are amplified by sqrt(H) in the scores dot, so plain
    fp16 W_a would be too coarse. The lo operand of the pairing matmul is
    h/2^11 (host-prescaled) so both partial products accumulate into one
    fp32 PSUM group. W_c is fp16 (tolerance 2e-2).

Device per core (PSUM accumulates fp32 everywhere):
  qa rows = h^T [W_a_hi | W_a_lo]              (PE fp16, streams W_a halves)
  qa^T via 8 tiny PE transposes                (PE fp16)
  per batch b:
    scores = qa_b . E_b^T                      (PE fp16, streams E^T)
    softmax on scores (1,1024) @ partition 0   (DVE max / ACT exp+sum / DVE)
    w = softmax * gauss / Z on window          (DVE fused, then fp16 copy)
    w^T via K=1 matmul with ones               (PE)
    context^T = eWin-chunks^T @ w^T            (PE fp16, 8 tiny matmuls)
  OUT = tanh([context; output] @ W_c^T)        (PE fp16 batched over 8; the
    output@W_c2 half is accumulated mid-stream, context@W_c1 at the tail)
"""

import os
import sys

import numpy as np

for _p in ("/opt/trn_rl_repo", "/root/.axon_site/_ro/trn_rl_repo"):
    if os.path.isdir(_p) and _p not in sys.path:
        sys.path.insert(0, _p)

N, L, H = 64, 1024, 1024
NCORES = 8
NB = N // NCORES  # batches per core
WIN = 128         # static window width after roll
DEV_POW = 25.0
KC = H // 128     # 8 contraction chunks
LO_SCALE = 2.0 ** 11

_PROGRAM = None


def _build_program():
    import concourse.tile as tile
    from concourse import bacc, mybir
    from concourse.bass import MemorySpace, ts
    from concourse.masks import make_identity
    from contextlib import ExitStack

    F32 = mybir.dt.float32
    F16 = mybir.dt.float16
    AF = mybir.ActivationFunctionType
    ALU = mybir.AluOpType

    nc = bacc.Bacc("TRN2", target_bir_lowering=False, debug=False, num_devices=NCORES)
    # eT pre-interleaved on host: [b, p, c*L+l] = E^T[b][128*c+p, l]
    # so every DMA is one contiguous 16KB read per partition.
    eT = nc.dram_tensor("eT", [NB, 128, KC * L], F16, kind="ExternalInput").ap()
    ewin = nc.dram_tensor("ewin", [NB, WIN, H], F16, kind="ExternalInput").ap()
    gauss = nc.dram_tensor("gauss", [1, NB * WIN], F32, kind="ExternalInput").ap()
    # outT16 = fp16(h); outTlo = fp16(2^-11 h) pairs with wa2's lo plane so the
    # lo partial products accumulate into the same PSUM group as the hi ones.
    outT16 = nc.dram_tensor("outT16", [H, NB], F16, kind="ExternalInput").ap()
    outTlo = nc.dram_tensor("outTlo", [H, NB], F16, kind="ExternalInput").ap()
    # wa2[hh, p, c, {hi,lo}, u] = W_pair[128c + p, 512hh + u]
    wa2 = nc.dram_tensor("wa2", [2, 128, KC, 2, 512], F16, kind="ExternalInput").ap()
    wcT = nc.dram_tensor("wcT", [128, 2 * KC, H], F16, kind="ExternalInput").ap()
    res = nc.dram_tensor("res", [NB, H], F32, kind="ExternalOutput").ap()

    with tile.TileContext(nc) as tc, ExitStack() as ctx:
        consts = ctx.enter_context(tc.tile_pool(name="consts", bufs=1))
        etp = ctx.enter_context(tc.tile_pool(name="etp", bufs=3))
        wap = ctx.enter_context(tc.tile_pool(name="wap", bufs=2))
        work = ctx.enter_context(tc.tile_pool(name="work", bufs=2))
        ps_s = ctx.enter_context(
            tc.tile_pool(name="ps_s", bufs=2, space=MemorySpace.PSUM)
        )
        ps_q = ctx.enter_context(
            tc.tile_pool(name="ps_q", bufs=1, space=MemorySpace.PSUM)
        )
        ps_o = ctx.enter_context(
            tc.tile_pool(name="ps_o", bufs=1, space=MemorySpace.PSUM)
        )
        ps_m = ctx.enter_context(
            tc.tile_pool(name="ps_m", bufs=1, space=MemorySpace.PSUM)
        )

        # ---- head DMAs: qa inputs first (critical path), then batch 0/1 ----
        outTr_sb = consts.tile([128, KC, NB], F16)
        nc.sync.dma_start(outTr_sb[:], outT16.rearrange("(c p) b -> p c b", p=128))
        outTlo_sb = consts.tile([128, KC, NB], F16)
        nc.sync.dma_start(outTlo_sb[:], outTlo.rearrange("(c p) b -> p c b", p=128))
        wa_tiles = []
        for hh in range(2):
            t = wap.tile([128, KC, 2, 512], F16, tag="wa")
            nc.sync.dma_start(t[:], wa2[hh])
            wa_tiles.append(t)
        et_tiles = {}
        for b in range(2):
            t = etp.tile([128, KC, L], F16, tag="et")
            nc.sync.dma_start(t[:], eT[b].rearrange("p (c l) -> p c l", l=L))
            et_tiles[b] = t
        # all 8 batch windows in one DMA: [l, b, h]
        ewin_sb = consts.tile([WIN, NB, H], F16)
        nc.sync.dma_start(ewin_sb[:], ewin.rearrange("b l h -> l b h"))
        gauss_sb = consts.tile([1, NB * WIN], F32)
        nc.sync.dma_start(gauss_sb[:], gauss[:])

        ident = consts.tile([128, 128], F16)
        make_identity(nc, ident[:])
        ones1 = consts.tile([1, 1], F16)
        nc.gpsimd.memset(ones1[:], 1.0)
        qaT_sb = consts.tile([128, KC, NB], F16)
        ctxAll = consts.tile([128, KC, NB], F16)

        # ---- qa rows = h^T W_a: hi and (pre-scaled) lo partial products all
        # accumulate into one fp32 PSUM group; one wa half-tile per hh ----
        qrow16 = consts.tile([NB, H], F16)
        for hh in range(2):
            ps_qa = ps_q.tile([NB, 512], F32, tag="q")
            for t in range(2):
                for c in range(KC):
                    nc.tensor.matmul(
                        ps_qa[:],
                        (outTr_sb if t == 0 else outTlo_sb)[:, c, :],
                        wa_tiles[hh][:, c, t, :],
                        start=(t == 0 and c == 0),
                        stop=(t == 1 and c == KC - 1),
                    )
            nc.vector.tensor_copy(qrow16[:, ts(hh, 512)], ps_qa[:])
            for cc in range(KC // 2):
                c = hh * (KC // 2) + cc
                ps_t = ps_m.tile([128, NB], F16, tag="misc")
                nc.tensor.transpose(ps_t[:], qrow16[:, ts(c, 128)], ident[0:NB, 0:NB])
                nc.vector.tensor_copy(qaT_sb[:, c, :], ps_t[:])

        # wcT quarters: h-half (d=8..15) first so its projection half can run
        # mid-stream; ctx-half (d=0..7) lands before the tail.
        wcT_sb = consts.tile([128, 2 * KC, H], F16)
        for q in (2, 3, 0, 1):
            nc.sync.dma_start(
                wcT_sb[:, ts(q, 2 * KC // 4), :], wcT[:, ts(q, 2 * KC // 4), :]
            )

        ps_out0 = ps_o.tile([NB, 512], F32, tag="out0")
        ps_out1 = ps_o.tile([NB, 512], F32, tag="out1")
        ps_out = [ps_out0, ps_out1]

        # ---- per-batch pipeline ----
        for b in range(NB):
            # prefetch batch b+2 (b0/b1 were issued before qa)
            pb = b + 2
            if pb < NB:
                t = etp.tile([128, KC, L], F16, tag="et")
                nc.sync.dma_start(t[:], eT[pb].rearrange("p (c l) -> p c l", l=L))
                et_tiles[pb] = t

            ps_scores = ps_s.tile([1, L], F32, tag="scores")
            et = et_tiles.pop(b)
            for c in range(KC):
                for hh in range(2):
                    nc.tensor.matmul(
                        ps_scores[:, ts(hh, 512)],
                        qaT_sb[:, c, b : b + 1],
                        et[:, c, ts(hh, 512)],
                        start=(c == 0),
                        stop=(c == KC - 1),
                    )

            negmax = work.tile([1, 1], F32, tag="negmax")
            nc.vector.reduce_max(
                negmax[:], ps_scores[:], axis=mybir.AxisListType.X, negate=True
            )
            expv = work.tile([1, L], F32, tag="expv")
            zsum = work.tile([1, 1], F32, tag="zsum")
            nc.scalar.activation(
                expv[:], ps_scores[:], AF.Exp, bias=negmax[:], accum_out=zsum[:]
            )
            rz = work.tile([1, 1], F32, tag="rz")
            nc.vector.reciprocal(rz[:], zsum[:])
            # w (window only) = exp * (1/Z) * gauss
            wv32 = work.tile([1, WIN], F32, tag="wv32")
            nc.vector.scalar_tensor_tensor(
                wv32[:],
                expv[:, 0:WIN],
                rz[:],
                gauss_sb[:, ts(b, WIN)],
                op0=ALU.mult,
                op1=ALU.mult,
            )
            wv16 = work.tile([1, WIN], F16, tag="wv16")
            nc.vector.tensor_copy(wv16[:], wv32[:])
            # w^T via K=1 matmul with ones: out = wv16^T
            ps_wT = ps_m.tile([128, 1], F32, tag="misc")
            nc.tensor.matmul(ps_wT[:], wv16[:], ones1[:], start=True, stop=True)
            wT_sb = work.tile([128, 1], F16, tag="wT")
            nc.vector.tensor_copy(wT_sb[:], ps_wT[:])
            # context^T chunks: eWin[:, b, ts(c,128)]^T @ w^T -> (128 h, 1)
            ps_ctx = ps_m.tile([128, NB], F32, tag="misc")
            for c in range(KC):
                nc.tensor.matmul(
                    ps_ctx[:, c : c + 1],
                    ewin_sb[:, b, ts(c, 128)],
                    wT_sb[:],
                    start=True,
                    stop=True,
                )
            nc.vector.tensor_copy(ctxAll[:, :, b], ps_ctx[:])

            if 3 <= b <= 6:
                # projection h-half: out += h @ W_c[:, H:]^T (no ctx dependency);
                # spread 4 matmuls per batch into the PE's slack
                for hh in range(2):
                    for dd in range(2):
                        d = KC + (b - 3) * 2 + dd
                        nc.tensor.matmul(
                            ps_out[hh][:],
                            outTr_sb[:, d - KC, :],
                            wcT_sb[:, d, ts(hh, 512)],
                            start=(d == KC),
                            stop=False,
                        )

        # ---- tail: out += ctx @ W_c[:, :H]^T, then tanh ----
        res_sb = work.tile([NB, H], F32, tag="res")
        for hh in range(2):
            for d in range(KC):
                nc.tensor.matmul(
                    ps_out[hh][:],
                    ctxAll[:, d, :],
                    wcT_sb[:, d, ts(hh, 512)],
                    start=False,
                    stop=(d == KC - 1),
                )
            nc.scalar.activation(res_sb[:, ts(hh, 512)], ps_out[hh][:], AF.Tanh)
        nc.sync.dma_start(res[:], res_sb[:])

    nc.compile()
    return nc


def _get_program():
    global _PROGRAM
    if _PROGRAM is None:
        _PROGRAM = _build_program()
    return _PROGRAM


def _prepare(inputs):
    E = np.asarray(inputs["encoder_outputs"], dtype=np.float32)
    out = np.asarray(inputs["output"], dtype=np.float32).reshape(N, H)
    W_a = np.ascontiguousarray(np.asarray(inputs["W_a"], dtype=np.float32))
    W_c = np.asarray(inputs["W_c"], dtype=np.float32)
    src_len = np.asarray(inputs["src_len"]).reshape(N).astype(np.int64)
    t = int(np.asarray(inputs["time_step"]))

    p_t = np.maximum(src_len - t, -1)
    roll = p_t - (WIN // 2 - 1)  # window slot j <-> original l = (j + roll) % L
    j = np.arange(L, dtype=np.int64)
    idx = (j[None, :] + roll[:, None]) % L  # (N, L)
    ptf = p_t.astype(np.float32)[:, None]
    gauss = np.exp(
        -((idx[:, :WIN].astype(np.float32) - ptf) ** 2) / np.float32(DEV_POW)
    ).astype(np.float32)  # (N, WIN)

    Er = E[np.arange(N)[:, None], idx, :]  # (N, L, H) rolled
    ewin_dev = np.ascontiguousarray(Er[:, :WIN, :]).astype(np.float16)  # (N, WIN, H)
    eT = np.ascontiguousarray(Er.transpose(0, 2, 1)).astype(np.float16)  # (N, H, L)
    # interleave for linear per-partition DMA: [n, p, c, l] = eT[n, 128c+p, l]
    eT_dev = np.ascontiguousarray(
        eT.reshape(N, KC, 128, L).transpose(0, 2, 1, 3)
    ).reshape(N, 128, KC * L)
    # W_a fp16 hi/lo pair: W ~= hi + 2^-11 * lo (lo scaled into fp16 range)
    wa_hi = W_a.astype(np.float16)
    wa_lo = ((W_a - wa_hi.astype(np.float32)) * np.float32(LO_SCALE)).astype(np.float16)
    # wa2[hh, p, c, t, u] = pair_t[128c + p, 512hh + u]
    wa2_dev = np.ascontiguousarray(
        np.stack([wa_hi, wa_lo], axis=1)  # (H, 2, H)
        .reshape(KC, 128, 2, 2, 512)
        .transpose(3, 1, 0, 2, 4)
    )
    wcT = np.ascontiguousarray(W_c.T)  # (2H, H)
    wcT_dev = np.ascontiguousarray(
        wcT.reshape(2 * KC, 128, H).transpose(1, 0, 2)
    ).astype(np.float16)  # (128, 2KC, H)

    in_maps = []
    for c in range(NCORES):
        sl = slice(c * NB, (c + 1) * NB)
        outT = np.ascontiguousarray(out[sl].T)
        in_maps.append(
            {
                "eT": eT_dev[sl],
                "ewin": ewin_dev[sl],
                "gauss": np.ascontiguousarray(gauss[sl].reshape(1, NB * WIN)),
                "outT16": outT.astype(np.float16),
                "outTlo": (outT / np.float32(LO_SCALE)).astype(np.float16),
                "wa2": wa2_dev,
                "wcT": wcT_dev,
            }
        )
    return in_maps


def _run(inputs, trace=False, tmpdir=None):
    from concourse.bass_utils import run_bass_kernel_spmd

    nc = _get_program()
    in_maps = _prepare(inputs)
    r = run_bass_kernel_spmd(
        nc, in_maps, core_ids=list(range(NCORES)), trace=trace, tmpdir=tmpdir
    )
    outp = np.concatenate([r.results[c]["res"] for c in range(NCORES)], axis=0)
    return np.ascontiguousarray(outp.reshape(N, 1, H).astype(np.float32)), r


def kernel(**inputs):
    return _run(inputs, trace=False)[0]


# revision 25
# speedup vs baseline: 2.5655x; 1.0030x over previous
"""Trainium2 Bass kernel for nn_LocalAttention (Luong local attention, N=64, L=H=1024).

Strategy
--------
Data-parallel over batch: 8 batches per NeuronCore x 8 cores.

Host-side layout prep (no model FLOPs on host):
  * For each batch n, p_t = max(src_len - time_step, -1). The Gaussian
    exp(-(l-p_t)^2/25) underflows to exactly 0.0f for |l-p_t| > 51, so the
    context reduction only needs a 128-wide window around p_t.
  * We ROLL each batch's source axis so that window lands at static slots
    [0, 128). Softmax (max/sum) is permutation-invariant, so scores/softmax
    computed in rolled coordinates are exact. Host passes rolled, transposed
    E^T (h on partitions) for the scores contraction, plus the first 128
    rolled rows as-is (eWin, l on partitions) for the context contraction.
  * W_a is passed as an fp16 hi/lo pair (W ~= hi + 2^-11 lo to ~22 mantissa
    bits): qa errors are amplified by sqrt(H) in the scores dot, so plain
    fp16 W_a would be too coarse. The lo operand of the pairing matmul is
    h/2^11 (host-prescaled) so both partial products accumulate into one
    fp32 PSUM group. W_c is fp16 (tolerance 2e-2).

Device per core (PSUM accumulates fp32 everywhere):
  qa rows = h^T [W_a_hi | W_a_lo]              (PE fp16, streams W_a halves)
  qa^T via 8 tiny PE transposes                (PE fp16)
  per batch b:
    scores = qa_b . E_b^T                      (PE fp16, streams E^T)
    softmax on scores (1,1024) @ partition 0   (DVE max / ACT exp+sum / DVE)
    w = softmax * gauss / Z on window          (DVE fused, then fp16 copy)
    w^T via K=1 matmul with ones               (PE)
    context^T = eWin-chunks^T @ w^T            (PE fp16, 8 tiny matmuls)
  OUT = tanh([context; output] @ W_c^T)        (PE fp16 batched over 8; the
    output@W_c2 half is accumulated mid-stream, context@W_c1 at the tail)
"""

import os
import sys

import numpy as np

for _p in ("/opt/trn_rl_repo", "/root/.axon_site/_ro/trn_rl_repo"):
    if os.path.isdir(_p) and _p not in sys.path:
        sys.path.insert(0, _p)

N, L, H = 64, 1024, 1024
NCORES = 8
NB = N // NCORES  # batches per core
WIN = 128         # static window width after roll
DEV_POW = 25.0
KC = H // 128     # 8 contraction chunks
LO_SCALE = 2.0 ** 11

_PROGRAM = None


def _build_program():
    import concourse.tile as tile
    from concourse import bacc, mybir
    from concourse.bass import MemorySpace, ts
    from concourse.masks import make_identity
    from contextlib import ExitStack

    F32 = mybir.dt.float32
    F16 = mybir.dt.float16
    AF = mybir.ActivationFunctionType
    ALU = mybir.AluOpType

    nc = bacc.Bacc("TRN2", target_bir_lowering=False, debug=False, num_devices=NCORES)
    # eT pre-interleaved on host: [b, p, c*L+l] = E^T[b][128*c+p, l]
    # so every DMA is one contiguous 16KB read per partition.
    eT = nc.dram_tensor("eT", [NB, 128, KC * L], F16, kind="ExternalInput").ap()
    ewin = nc.dram_tensor("ewin", [NB, WIN, H], F16, kind="ExternalInput").ap()
    gauss = nc.dram_tensor("gauss", [1, NB * WIN], F32, kind="ExternalInput").ap()
    # outT16 = fp16(h); outTlo = fp16(2^-11 h) pairs with wa2's lo plane so the
    # lo partial products accumulate into the same PSUM group as the hi ones.
    outT16 = nc.dram_tensor("outT16", [H, NB], F16, kind="ExternalInput").ap()
    outTlo = nc.dram_tensor("outTlo", [H, NB], F16, kind="ExternalInput").ap()
    # wa2[hh, p, c, {hi,lo}, u] = W_pair[128c + p, 512hh + u]
    wa2 = nc.dram_tensor("wa2", [2, 128, KC, 2, 512], F16, kind="ExternalInput").ap()
    wcT = nc.dram_tensor("wcT", [128, 2 * KC, H], F16, kind="ExternalInput").ap()
    res = nc.dram_tensor("res", [NB, H], F32, kind="ExternalOutput").ap()

    with tile.TileContext(nc) as tc, ExitStack() as ctx:
        consts = ctx.enter_context(tc.tile_pool(name="consts", bufs=1))
        etp = ctx.enter_context(tc.tile_pool(name="etp", bufs=4))
        ewp = ctx.enter_context(tc.tile_pool(name="ewp", bufs=4))
        wap = ctx.enter_context(tc.tile_pool(name="wap", bufs=2))
        work = ctx.enter_context(tc.tile_pool(name="work", bufs=2))
        ps_s = ctx.enter_context(
            tc.tile_pool(name="ps_s", bufs=2, space=MemorySpace.PSUM)
        )
        ps_q = ctx.enter_context(
            tc.tile_pool(name="ps_q", bufs=1, space=MemorySpace.PSUM)
        )
        ps_o = ctx.enter_context(
            tc.tile_pool(name="ps_o", bufs=1, space=MemorySpace.PSUM)
        )
        ps_m = ctx.enter_context(
            tc.tile_pool(name="ps_m", bufs=1, space=MemorySpace.PSUM)
        )

        # ---- head DMAs: qa inputs first (critical path), then batch 0/1 ----
        outTr_sb = consts.tile([128, KC, NB], F16)
        nc.sync.dma_start(outTr_sb[:], outT16.rearrange("(c p) b -> p c b", p=128))
        outTlo_sb = consts.tile([128, KC, NB], F16)
        nc.sync.dma_start(outTlo_sb[:], outTlo.rearrange("(c p) b -> p c b", p=128))
        wa_tiles = []
        for hh in range(2):
            t = wap.tile([128, KC, 2, 512], F16, tag="wa")
            nc.sync.dma_start(t[:], wa2[hh])
            wa_tiles.append(t)
        gauss_sb = consts.tile([1, NB * WIN], F32)
        nc.sync.dma_start(gauss_sb[:], gauss[:])
        et_tiles = {}
        ewin_tiles = {}
        for b in range(2):
            t = etp.tile([128, KC, L], F16, tag="et")
            nc.sync.dma_start(t[:], eT[b].rearrange("p (c l) -> p c l", l=L))
            et_tiles[b] = t
            t = ewp.tile([WIN, H], F16, tag="ewin")
            nc.sync.dma_start(t[:], ewin[b])
            ewin_tiles[b] = t

        ident = consts.tile([128, 128], F16)
        make_identity(nc, ident[:])
        ones1 = consts.tile([1, 1], F16)
        nc.gpsimd.memset(ones1[:], 1.0)
        qaT_sb = consts.tile([128, KC, NB], F16)
        ctxAll = consts.tile([128, KC, NB], F16)

        # ---- qa rows = h^T W_a: hi and (pre-scaled) lo partial products all
        # accumulate into one fp32 PSUM group; one wa half-tile per hh ----
        qrow16 = consts.tile([NB, H], F16)
        for hh in range(2):
            ps_qa = ps_q.tile([NB, 512], F32, tag="q")
            for t in range(2):
                for c in range(KC):
                    nc.tensor.matmul(
                        ps_qa[:],
                        (outTr_sb if t == 0 else outTlo_sb)[:, c, :],
                        wa_tiles[hh][:, c, t, :],
                        start=(t == 0 and c == 0),
                        stop=(t == 1 and c == KC - 1),
                    )
            nc.vector.tensor_copy(qrow16[:, ts(hh, 512)], ps_qa[:])
            for cc in range(KC // 2):
                c = hh * (KC // 2) + cc
                ps_t = ps_m.tile([128, NB], F16, tag="misc")
                nc.tensor.transpose(ps_t[:], qrow16[:, ts(c, 128)], ident[0:NB, 0:NB])
                nc.vector.tensor_copy(qaT_sb[:, c, :], ps_t[:])

        wcT_sb = consts.tile([128, 2 * KC, H], F16)

        ps_out0 = ps_o.tile([NB, 512], F32, tag="out0")
        ps_out1 = ps_o.tile([NB, 512], F32, tag="out1")
        ps_out = [ps_out0, ps_out1]

        def ctx_block(b, wv16, ew):
            """w^T transpose + context matmuls for batch b (PE + DVE)."""
            ps_wT = ps_m.tile([128, 1], F32, tag="misc")
            nc.tensor.matmul(ps_wT[:], wv16[:], ones1[:], start=True, stop=True)
            wT_sb = work.tile([128, 1], F16, tag="wT")
            nc.vector.tensor_copy(wT_sb[:], ps_wT[:])
            ps_ctx = ps_m.tile([128, NB], F32, tag="misc")
            for c in range(KC):
                nc.tensor.matmul(
                    ps_ctx[:, c : c + 1],
                    ew[:, ts(c, 128)],
                    wT_sb[:],
                    start=True,
                    stop=True,
                )
            nc.vector.tensor_copy(ctxAll[:, :, b], ps_ctx[:])

        # ---- per-batch pipeline: scores(b) issue ahead of ctx(b-1) so the
        # in-order PE queue never stalls on batch b-1's softmax chain ----
        pending = None  # (b, wv16, ewin_tile) awaiting ctx
        for b in range(NB):
            # prefetch batch b+2 (b0/b1 were issued before qa)
            pb = b + 2
            if pb < NB:
                t = etp.tile([128, KC, L], F16, tag="et")
                nc.sync.dma_start(t[:], eT[pb].rearrange("p (c l) -> p c l", l=L))
                et_tiles[pb] = t
                t = ewp.tile([WIN, H], F16, tag="ewin")
                nc.sync.dma_start(t[:], ewin[pb])
                ewin_tiles[pb] = t
            if 2 <= b <= 5:
                # wcT quarters trickled behind the eT stream: h-half (d=8..15)
                # first for the pre-tail projection, ctx-half before the tail.
                q = (2, 3, 0, 1)[b - 2]
                nc.sync.dma_start(
                    wcT_sb[:, ts(q, 2 * KC // 4), :], wcT[:, ts(q, 2 * KC // 4), :]
                )

            ps_scores = ps_s.tile([1, L], F32, tag="scores")
            et = et_tiles.pop(b)
            for c in range(KC):
                for hh in range(2):
                    nc.tensor.matmul(
                        ps_scores[:, ts(hh, 512)],
                        qaT_sb[:, c, b : b + 1],
                        et[:, c, ts(hh, 512)],
                        start=(c == 0),
                        stop=(c == KC - 1),
                    )

            negmax = work.tile([1, 1], F32, tag="negmax")
            nc.vector.reduce_max(
                negmax[:], ps_scores[:], axis=mybir.AxisListType.X, negate=True
            )
            expv = work.tile([1, L], F32, tag="expv")
            zsum = work.tile([1, 1], F32, tag="zsum")
            nc.scalar.activation(
                expv[:], ps_scores[:], AF.Exp, bias=negmax[:], accum_out=zsum[:]
            )
            rz = work.tile([1, 1], F32, tag="rz")
            nc.vector.reciprocal(rz[:], zsum[:])
            # w (window only) = exp * (1/Z) * gauss
            wv32 = work.tile([1, WIN], F32, tag="wv32")
            nc.vector.scalar_tensor_tensor(
                wv32[:],
                expv[:, 0:WIN],
                rz[:],
                gauss_sb[:, ts(b, WIN)],
                op0=ALU.mult,
                op1=ALU.mult,
            )
            wv16 = work.tile([1, WIN], F16, tag="wv16")
            nc.vector.tensor_copy(wv16[:], wv32[:])

            if pending is not None:
                ctx_block(*pending)
            pending = (b, wv16, ewin_tiles.pop(b))

        # projection h-half: out += h @ W_c[:, H:]^T (no ctx dependency) —
        # fills the PE while batch 7's softmax chain drains
        for hh in range(2):
            for d in range(KC, 2 * KC):
                nc.tensor.matmul(
                    ps_out[hh][:],
                    outTr_sb[:, d - KC, :],
                    wcT_sb[:, d, ts(hh, 512)],
                    start=(d == KC),
                    stop=False,
                )
        ctx_block(*pending)

        # ---- tail: out += ctx @ W_c[:, :H]^T, then tanh, res in two halves ----
        res_sb = work.tile([NB, H], F32, tag="res")
        for hh in range(2):
            for d in range(KC):
                nc.tensor.matmul(
                    ps_out[hh][:],
                    ctxAll[:, d, :],
                    wcT_sb[:, d, ts(hh, 512)],
                    start=False,
                    stop=(d == KC - 1),
                )
            nc.scalar.activation(res_sb[:, ts(hh, 512)], ps_out[hh][:], AF.Tanh)
            nc.sync.dma_start(res[:, ts(hh, 512)], res_sb[:, ts(hh, 512)])

    nc.compile()
    return nc


def _get_program():
    global _PROGRAM
    if _PROGRAM is None:
        _PROGRAM = _build_program()
    return _PROGRAM


def _prepare(inputs):
    E = np.asarray(inputs["encoder_outputs"], dtype=np.float32)
    out = np.asarray(inputs["output"], dtype=np.float32).reshape(N, H)
    W_a = np.ascontiguousarray(np.asarray(inputs["W_a"], dtype=np.float32))
    W_c = np.asarray(inputs["W_c"], dtype=np.float32)
    src_len = np.asarray(inputs["src_len"]).reshape(N).astype(np.int64)
    t = int(np.asarray(inputs["time_step"]))

    p_t = np.maximum(src_len - t, -1)
    roll = p_t - (WIN // 2 - 1)  # window slot j <-> original l = (j + roll) % L
    j = np.arange(L, dtype=np.int64)
    idx = (j[None, :] + roll[:, None]) % L  # (N, L)
    ptf = p_t.astype(np.float32)[:, None]
    gauss = np.exp(
        -((idx[:, :WIN].astype(np.float32) - ptf) ** 2) / np.float32(DEV_POW)
    ).astype(np.float32)  # (N, WIN)

    Er = E[np.arange(N)[:, None], idx, :]  # (N, L, H) rolled
    ewin_dev = np.ascontiguousarray(Er[:, :WIN, :]).astype(np.float16)  # (N, WIN, H)
    eT = np.ascontiguousarray(Er.transpose(0, 2, 1)).astype(np.float16)  # (N, H, L)
    # interleave for linear per-partition DMA: [n, p, c, l] = eT[n, 128c+p, l]
    eT_dev = np.ascontiguousarray(
        eT.reshape(N, KC, 128, L).transpose(0, 2, 1, 3)
    ).reshape(N, 128, KC * L)
    # W_a fp16 hi/lo pair: W ~= hi + 2^-11 * lo (lo scaled into fp16 range)
    wa_hi = W_a.astype(np.float16)
    wa_lo = ((W_a - wa_hi.astype(np.float32)) * np.float32(LO_SCALE)).astype(np.float16)
    # wa2[hh, p, c, t, u] = pair_t[128c + p, 512hh + u]
    wa2_dev = np.ascontiguousarray(
        np.stack([wa_hi, wa_lo], axis=1)  # (H, 2, H)
        .reshape(KC, 128, 2, 2, 512)
        .transpose(3, 1, 0, 2, 4)
    )
    wcT = np.ascontiguousarray(W_c.T)  # (2H, H)
    wcT_dev = np.ascontiguousarray(
        wcT.reshape(2 * KC, 128, H).transpose(1, 0, 2)
    ).astype(np.float16)  # (128, 2KC, H)

    in_maps = []
    for c in range(NCORES):
        sl = slice(c * NB, (c + 1) * NB)
        outT = np.ascontiguousarray(out[sl].T)
        in_maps.append(
            {
                "eT": eT_dev[sl],
                "ewin": ewin_dev[sl],
                "gauss": np.ascontiguousarray(gauss[sl].reshape(1, NB * WIN)),
                "outT16": outT.astype(np.float16),
                "outTlo": (outT / np.float32(LO_SCALE)).astype(np.float16),
                "wa2": wa2_dev,
                "wcT": wcT_dev,
            }
        )
    return in_maps


def _run(inputs, trace=False, tmpdir=None):
    from concourse.bass_utils import run_bass_kernel_spmd

    nc = _get_program()
    in_maps = _prepare(inputs)
    r = run_bass_kernel_spmd(
        nc, in_maps, core_ids=list(range(NCORES)), trace=trace, tmpdir=tmpdir
    )
    outp = np.concatenate([r.results[c]["res"] for c in range(NCORES)], axis=0)
    return np.ascontiguousarray(outp.reshape(N, 1, H).astype(np.float32)), r


def kernel(**inputs):
    return _run(inputs, trace=False)[0]


# revision 27
# speedup vs baseline: 2.6199x; 1.0212x over previous
"""Trainium2 Bass kernel for nn_LocalAttention (Luong local attention, N=64, L=H=1024).

Strategy
--------
Data-parallel over batch: 8 batches per NeuronCore x 8 cores.

Host-side layout prep (no model FLOPs on host):
  * For each batch n, p_t = max(src_len - time_step, -1). The Gaussian
    exp(-(l-p_t)^2/25) underflows to exactly 0.0f for |l-p_t| > 51, so the
    context reduction only needs a 128-wide window around p_t.
  * We ROLL each batch's source axis so that window lands at static slots
    [0, 128). Softmax (max/sum) is permutation-invariant, so scores/softmax
    computed in rolled coordinates are exact. Host passes rolled, transposed
    E^T (h on partitions) for the scores contraction, plus the first 128
    rolled rows as-is (eWin, l on partitions) for the context contraction.
  * W_a is passed as an fp16 hi/lo pair (W ~= hi + 2^-11 lo to ~22 mantissa
    bits): qa errors are amplified by sqrt(H) in the scores dot, so plain
    fp16 W_a would be too coarse. The lo operand of the pairing matmul is
    h/2^11 (host-prescaled) so both partial products accumulate into one
    fp32 PSUM group. W_c is fp16 (tolerance 2e-2).

Device per core (PSUM accumulates fp32 everywhere):
  qa rows = h^T [W_a_hi | W_a_lo]              (PE fp16, streams W_a halves)
  qa^T via 8 tiny PE transposes                (PE fp16)
  per batch b:
    scores = qa_b . E_b^T                      (PE fp16, streams E^T)
    softmax on scores (1,1024) @ partition 0   (DVE max / ACT exp+sum / DVE)
    w = softmax * gauss / Z on window          (DVE fused, then fp16 copy)
    w^T via K=1 matmul with ones               (PE)
    context^T = eWin-chunks^T @ w^T            (PE fp16, 8 tiny matmuls)
  OUT = tanh([context; output] @ W_c^T)        (PE fp16 batched over 8; the
    output@W_c2 half is accumulated mid-stream, context@W_c1 at the tail)
"""

import os
import sys

import numpy as np

for _p in ("/opt/trn_rl_repo", "/root/.axon_site/_ro/trn_rl_repo"):
    if os.path.isdir(_p) and _p not in sys.path:
        sys.path.insert(0, _p)

N, L, H = 64, 1024, 1024
NCORES = 8
NB = N // NCORES  # batches per core
WIN = 128         # static window width after roll
DEV_POW = 25.0
KC = H // 128     # 8 contraction chunks
LO_SCALE = 2.0 ** 11

_PROGRAM = None


def _build_program():
    import concourse.tile as tile
    from concourse import bacc, mybir
    from concourse.bass import MemorySpace, ts
    from concourse.masks import make_identity
    from contextlib import ExitStack

    F32 = mybir.dt.float32
    F16 = mybir.dt.float16
    AF = mybir.ActivationFunctionType
    ALU = mybir.AluOpType

    nc = bacc.Bacc("TRN2", target_bir_lowering=False, debug=False, num_devices=NCORES)
    # eT pre-interleaved on host: [b, p, c*L+l] = E^T[b][128*c+p, l]
    # so every DMA is one contiguous 16KB read per partition.
    eT = nc.dram_tensor("eT", [NB, 128, KC * L], F16, kind="ExternalInput").ap()
    ewin = nc.dram_tensor("ewin", [NB, WIN, H], F16, kind="ExternalInput").ap()
    gauss = nc.dram_tensor("gauss", [1, NB * WIN], F32, kind="ExternalInput").ap()
    # outT16 = fp16(h); outTlo = fp16(2^-11 h) pairs with wa2's lo plane so the
    # lo partial products accumulate into the same PSUM group as the hi ones.
    outT16 = nc.dram_tensor("outT16", [H, NB], F16, kind="ExternalInput").ap()
    outTlo = nc.dram_tensor("outTlo", [H, NB], F16, kind="ExternalInput").ap()
    # wa2[hh, p, c, {hi,lo}, u] = W_pair[128c + p, 512hh + u]
    wa2 = nc.dram_tensor("wa2", [2, 128, KC, 2, 512], F16, kind="ExternalInput").ap()
    wcT = nc.dram_tensor("wcT", [128, 2 * KC, H], F16, kind="ExternalInput").ap()
    res = nc.dram_tensor("res", [NB, H], F32, kind="ExternalOutput").ap()

    with tile.TileContext(nc) as tc, ExitStack() as ctx:
        consts = ctx.enter_context(tc.tile_pool(name="consts", bufs=1))
        etp = ctx.enter_context(tc.tile_pool(name="etp", bufs=4))
        ewp = ctx.enter_context(tc.tile_pool(name="ewp", bufs=4))
        wap = ctx.enter_context(tc.tile_pool(name="wap", bufs=2))
        work = ctx.enter_context(tc.tile_pool(name="work", bufs=2))
        ps_s = ctx.enter_context(
            tc.tile_pool(name="ps_s", bufs=2, space=MemorySpace.PSUM)
        )
        ps_q = ctx.enter_context(
            tc.tile_pool(name="ps_q", bufs=1, space=MemorySpace.PSUM)
        )
        ps_o = ctx.enter_context(
            tc.tile_pool(name="ps_o", bufs=1, space=MemorySpace.PSUM)
        )
        ps_m = ctx.enter_context(
            tc.tile_pool(name="ps_m", bufs=1, space=MemorySpace.PSUM)
        )

        # ---- head DMAs: qa inputs first (critical path), then batch 0/1 ----
        outTr_sb = consts.tile([128, KC, NB], F16)
        nc.sync.dma_start(outTr_sb[:], outT16.rearrange("(c p) b -> p c b", p=128))
        outTlo_sb = consts.tile([128, KC, NB], F16)
        nc.sync.dma_start(outTlo_sb[:], outTlo.rearrange("(c p) b -> p c b", p=128))
        wa_tiles = []
        for hh in range(2):
            t = wap.tile([128, KC, 2, 512], F16, tag="wa")
            nc.sync.dma_start(t[:], wa2[hh])
            wa_tiles.append(t)
        gauss_sb = consts.tile([1, NB * WIN], F32)
        nc.sync.dma_start(gauss_sb[:], gauss[:])
        et_tiles = {}
        ewin_tiles = {}
        for b in range(2):
            t = etp.tile([128, KC, L], F16, tag="et")
            nc.sync.dma_start(t[:], eT[b].rearrange("p (c l) -> p c l", l=L))
            et_tiles[b] = t
            t = ewp.tile([WIN, H], F16, tag="ewin")
            nc.sync.dma_start(t[:], ewin[b])
            ewin_tiles[b] = t

        ident = consts.tile([128, 128], F16)
        make_identity(nc, ident[:])
        ones1 = consts.tile([1, 1], F16)
        nc.gpsimd.memset(ones1[:], 1.0)
        qaT_sb = consts.tile([128, KC, NB], F16)
        ctxAll = consts.tile([128, KC, NB], F16)

        # ---- qa rows = h^T W_a: hi and (pre-scaled) lo partial products all
        # accumulate into one fp32 PSUM group; one wa half-tile per hh ----
        qrow16 = consts.tile([NB, H], F16)
        for hh in range(2):
            ps_qa = ps_q.tile([NB, 512], F32, tag="q")
            for t in range(2):
                for c in range(KC):
                    nc.tensor.matmul(
                        ps_qa[:],
                        (outTr_sb if t == 0 else outTlo_sb)[:, c, :],
                        wa_tiles[hh][:, c, t, :],
                        start=(t == 0 and c == 0),
                        stop=(t == 1 and c == KC - 1),
                    )
            nc.vector.tensor_copy(qrow16[:, ts(hh, 512)], ps_qa[:])
            for cc in range(KC // 2):
                c = hh * (KC // 2) + cc
                ps_t = ps_m.tile([128, NB], F16, tag="misc")
                nc.tensor.transpose(ps_t[:], qrow16[:, ts(c, 128)], ident[0:NB, 0:NB])
                nc.vector.tensor_copy(qaT_sb[:, c, :], ps_t[:])

        wcT_sb = consts.tile([128, 2 * KC, H], F16)

        ps_out0 = ps_o.tile([NB, 512], F32, tag="out0")
        ps_out1 = ps_o.tile([NB, 512], F32, tag="out1")
        ps_out = [ps_out0, ps_out1]

        def ctx_block(b, wv16, ew):
            """w^T transpose + context matmuls for batch b (PE + DVE)."""
            ps_wT = ps_m.tile([128, 1], F32, tag="misc")
            nc.tensor.matmul(ps_wT[:], wv16[:], ones1[:], start=True, stop=True)
            wT_sb = work.tile([128, 1], F16, tag="wT")
            nc.vector.tensor_copy(wT_sb[:], ps_wT[:])
            ps_ctx = ps_m.tile([128, NB], F32, tag="misc")
            for c in range(KC):
                nc.tensor.matmul(
                    ps_ctx[:, c : c + 1],
                    ew[:, ts(c, 128)],
                    wT_sb[:],
                    start=True,
                    stop=True,
                )
            nc.vector.tensor_copy(ctxAll[:, :, b], ps_ctx[:])

        # ---- per-batch pipeline: scores(b) issue ahead of ctx(b-1) so the
        # in-order PE queue never stalls on batch b-1's softmax chain ----
        pending = None  # (b, wv16, ewin_tile) awaiting ctx
        for b in range(NB):
            # prefetch batch b+2 (b0/b1 were issued before qa)
            pb = b + 2
            if pb < NB:
                t = etp.tile([128, KC, L], F16, tag="et")
                nc.sync.dma_start(t[:], eT[pb].rearrange("p (c l) -> p c l", l=L))
                et_tiles[pb] = t
                t = ewp.tile([WIN, H], F16, tag="ewin")
                nc.sync.dma_start(t[:], ewin[pb])
                ewin_tiles[pb] = t
            if 2 <= b <= 5:
                # wcT quarters trickled behind the eT stream: h-half (d=8..15)
                # first for the pre-tail projection, ctx-half before the tail.
                q = (2, 3, 0, 1)[b - 2]
                nc.sync.dma_start(
                    wcT_sb[:, ts(q, 2 * KC // 4), :], wcT[:, ts(q, 2 * KC // 4), :]
                )

            ps_scores = ps_s.tile([1, L], F32, tag="scores")
            et = et_tiles.pop(b)
            for c in range(KC):
                for hh in range(2):
                    nc.tensor.matmul(
                        ps_scores[:, ts(hh, 512)],
                        qaT_sb[:, c, b : b + 1],
                        et[:, c, ts(hh, 512)],
                        start=(c == 0),
                        stop=(c == KC - 1),
                    )

            # batch b-1's ctx work goes ahead of batch b's softmax ops so the
            # in-order DVE queue can't stall it behind exp(b)
            if pending is not None:
                ctx_block(*pending)
                pending = None

            negmax = work.tile([1, 1], F32, tag="negmax")
            nc.vector.reduce_max(
                negmax[:], ps_scores[:], axis=mybir.AxisListType.X, negate=True
            )
            expv = work.tile([1, L], F32, tag="expv")
            zsum = work.tile([1, 1], F32, tag="zsum")
            nc.scalar.activation(
                expv[:], ps_scores[:], AF.Exp, bias=negmax[:], accum_out=zsum[:]
            )
            rz = work.tile([1, 1], F32, tag="rz")
            nc.vector.reciprocal(rz[:], zsum[:])
            # w (window only) = exp * (1/Z) * gauss
            wv32 = work.tile([1, WIN], F32, tag="wv32")
            nc.vector.scalar_tensor_tensor(
                wv32[:],
                expv[:, 0:WIN],
                rz[:],
                gauss_sb[:, ts(b, WIN)],
                op0=ALU.mult,
                op1=ALU.mult,
            )
            wv16 = work.tile([1, WIN], F16, tag="wv16")
            nc.vector.tensor_copy(wv16[:], wv32[:])
            pending = (b, wv16, ewin_tiles.pop(b))

        # projection h-half: out += h @ W_c[:, H:]^T (no ctx dependency) —
        # fills the PE while batch 7's softmax chain drains
        for hh in range(2):
            for d in range(KC, 2 * KC):
                nc.tensor.matmul(
                    ps_out[hh][:],
                    outTr_sb[:, d - KC, :],
                    wcT_sb[:, d, ts(hh, 512)],
                    start=(d == KC),
                    stop=False,
                )
        ctx_block(*pending)

        # ---- tail: out += ctx @ W_c[:, :H]^T, then tanh, res in two halves ----
        res_sb = work.tile([NB, H], F32, tag="res")
        for hh in range(2):
            for d in range(KC):
                nc.tensor.matmul(
                    ps_out[hh][:],
                    ctxAll[:, d, :],
                    wcT_sb[:, d, ts(hh, 512)],
                    start=False,
                    stop=(d == KC - 1),
                )
            nc.scalar.activation(res_sb[:, ts(hh, 512)], ps_out[hh][:], AF.Tanh)
            nc.sync.dma_start(res[:, ts(hh, 512)], res_sb[:, ts(hh, 512)])

    nc.compile()
    return nc


def _get_program():
    global _PROGRAM
    if _PROGRAM is None:
        _PROGRAM = _build_program()
    return _PROGRAM


def _prepare(inputs):
    E = np.asarray(inputs["encoder_outputs"], dtype=np.float32)
    out = np.asarray(inputs["output"], dtype=np.float32).reshape(N, H)
    W_a = np.ascontiguousarray(np.asarray(inputs["W_a"], dtype=np.float32))
    W_c = np.asarray(inputs["W_c"], dtype=np.float32)
    src_len = np.asarray(inputs["src_len"]).reshape(N).astype(np.int64)
    t = int(np.asarray(inputs["time_step"]))

    p_t = np.maximum(src_len - t, -1)
    roll = p_t - (WIN // 2 - 1)  # window slot j <-> original l = (j + roll) % L
    j = np.arange(L, dtype=np.int64)
    idx = (j[None, :] + roll[:, None]) % L  # (N, L)
    ptf = p_t.astype(np.float32)[:, None]
    gauss = np.exp(
        -((idx[:, :WIN].astype(np.float32) - ptf) ** 2) / np.float32(DEV_POW)
    ).astype(np.float32)  # (N, WIN)

    Er = E[np.arange(N)[:, None], idx, :]  # (N, L, H) rolled
    ewin_dev = np.ascontiguousarray(Er[:, :WIN, :]).astype(np.float16)  # (N, WIN, H)
    eT = np.ascontiguousarray(Er.transpose(0, 2, 1)).astype(np.float16)  # (N, H, L)
    # interleave for linear per-partition DMA: [n, p, c, l] = eT[n, 128c+p, l]
    eT_dev = np.ascontiguousarray(
        eT.reshape(N, KC, 128, L).transpose(0, 2, 1, 3)
    ).reshape(N, 128, KC * L)
    # W_a fp16 hi/lo pair: W ~= hi + 2^-11 * lo (lo scaled into fp16 range)
    wa_hi = W_a.astype(np.float16)
    wa_lo = ((W_a - wa_hi.astype(np.float32)) * np.float32(LO_SCALE)).astype(np.float16)
    # wa2[hh, p, c, t, u] = pair_t[128c + p, 512hh + u]
    wa2_dev = np.ascontiguousarray(
        np.stack([wa_hi, wa_lo], axis=1)  # (H, 2, H)
        .reshape(KC, 128, 2, 2, 512)
        .transpose(3, 1, 0, 2, 4)
    )
    wcT = np.ascontiguousarray(W_c.T)  # (2H, H)
    wcT_dev = np.ascontiguousarray(
        wcT.reshape(2 * KC, 128, H).transpose(1, 0, 2)
    ).astype(np.float16)  # (128, 2KC, H)

    in_maps = []
    for c in range(NCORES):
        sl = slice(c * NB, (c + 1) * NB)
        outT = np.ascontiguousarray(out[sl].T)
        in_maps.append(
            {
                "eT": eT_dev[sl],
                "ewin": ewin_dev[sl],
                "gauss": np.ascontiguousarray(gauss[sl].reshape(1, NB * WIN)),
                "outT16": outT.astype(np.float16),
                "outTlo": (outT / np.float32(LO_SCALE)).astype(np.float16),
                "wa2": wa2_dev,
                "wcT": wcT_dev,
            }
        )
    return in_maps


def _run(inputs, trace=False, tmpdir=None):
    from concourse.bass_utils import run_bass_kernel_spmd

    nc = _get_program()
    in_maps = _prepare(inputs)
    r = run_bass_kernel_spmd(
        nc, in_maps, core_ids=list(range(NCORES)), trace=trace, tmpdir=tmpdir
    )
    outp = np.concatenate([r.results[c]["res"] for c in range(NCORES)], axis=0)
    return np.ascontiguousarray(outp.reshape(N, 1, H).astype(np.float32)), r


def kernel(**inputs):
    return _run(inputs, trace=False)[0]
